# revision 3
# baseline (speedup 1.0000x reference)
"""Trainium2 Bass kernel for nn_NaryDecoderCell (batched per-degree GRU decoder).

Sharding: expert-parallel over the degree dim D=8 -> one degree slot per
NeuronCore. Each core keeps its degree's weights resident in SBUF and streams
the full batch through. Host-side unshard: probs columns are gathered, and the
per-degree hc partial sums are reduced + tanh'd.

Math per degree d (reference semantics):
  h0 = concat(parent_h, encoding)                  [B, 1024]
  gi = x @ W_ih[d].T + b_ih[d]                     [B, 3072]
  gh = h0 @ W_hh[d].T + b_hh[d]                    [B, 3072]
  r = sigmoid(gi_r + gh_r); z = sigmoid(gi_z + gh_z)
  n = tanh(gi_n + r * gh_n)
  hn = (1-z)*n + z*h0
  probs_d = sigmoid(hn @ W_p[d] + b_p[d])          [B]
  hc_d = hn @ W_h[d].T + b_h[d]                    [B, 512]
Output: h = tanh(sum_d hc_d) [B, 512], probs [B, 8].

Device layout: everything is kept transposed (feature dim on partitions,
batch on the free axis) so both matmul chains contract along partitions.
The rhs stack R = concat(xT, h0T) [1280, B] gives 10 uniform K-tiles; gate
weights are host-reordered to [1280, 8(m), 384(r|z|n)] so that the r/z gates
accumulate gi+gh directly in PSUM and the per-m slices are contiguous.
Matmuls run as float32r (full PE rate at N>=256, fp32 data).
"""

import sys

sys.path.insert(0, "/opt/trn_rl_repo")

import numpy as np

B = 2048
C = 256
HS = 512
H2 = 2 * HS          # 1024
D = 8
P = 128
BCH = 256            # batch chunk (matmul moving free dim)
NCH = B // BCH       # 8
KT = (C + H2) // P   # 10 rhs K-tiles (2 from xT, 8 from h0T)
KH = H2 // P         # 8  hn K-tiles
MT = H2 // P         # 8  gate row tiles per gate

_CACHE = {}


def _build():
    import concourse.bacc as bacc
    import concourse.mybir as mybir
    from concourse.tile import TileContext

    F32 = mybir.dt.float32
    F32R = mybir.dt.float32r
    AFT = mybir.ActivationFunctionType
    ALU = mybir.AluOpType

    nc = bacc.Bacc("TRN2", target_bir_lowering=False, debug=False,
                   enable_asserts=False, num_devices=D)

    R = nc.dram_tensor("R", [C + H2, B], F32R, kind="ExternalInput").ap()
    Wg = nc.dram_tensor("Wg", [C + H2, MT, 384], F32R, kind="ExternalInput").ap()
    Wh2 = nc.dram_tensor("Wh2", [H2, HS], F32R, kind="ExternalInput").ap()
    Wp2 = nc.dram_tensor("Wp2", [P, KH], F32R, kind="ExternalInput").ap()
    Bv = nc.dram_tensor("Bv", [P, 37], F32, kind="ExternalInput").ap()
    hc_out = nc.dram_tensor("hc_out", [HS, B], F32, kind="ExternalOutput").ap()
    probs_out = nc.dram_tensor("probs_out", [1, B], F32, kind="ExternalOutput").ap()

    with TileContext(nc) as tc:
        with (
            tc.tile_pool(name="w", bufs=1) as wp,
            tc.tile_pool(name="acts", bufs=2) as ap_,
            tc.tile_pool(name="hnp", bufs=1) as hp,
            tc.tile_pool(name="tmp", bufs=2) as tp,
            tc.tile_pool(name="outs", bufs=3) as op_,
            tc.tile_pool(name="pg", bufs=1, space="PSUM") as pg,
            tc.tile_pool(name="po", bufs=1, space="PSUM") as po,
        ):
            # ---- resident weights ----
            wg_t = [wp.tile([P, MT, 384], F32R, tag=f"wg{k}", name=f"wg{k}")
                    for k in range(KT)]
            wh_t = [wp.tile([P, HS], F32R, tag=f"wh{k}", name=f"wh{k}")
                    for k in range(KH)]
            wpv = wp.tile([P, KH], F32R, name="wpv")
            bias = wp.tile([P, 37], F32, name="bias")
            nc.sync.dma_start(bias[:], Bv[:])
            nc.sync.dma_start(wpv[:], Wp2[:])
            # m-major so the first m iteration's weights land first
            for m in range(MT):
                for k in range(KT):
                    nc.sync.dma_start(wg_t[k][:, m, :], Wg[k * P:(k + 1) * P, m, :])
            for k in range(KH):
                nc.sync.dma_start(wh_t[k][:], Wh2[k * P:(k + 1) * P, :])

            for ch in range(NCH):
                cs = slice(ch * BCH, (ch + 1) * BCH)
                r_t = [ap_.tile([P, BCH], F32R, tag=f"rhs{k}", name=f"rhs{k}")
                       for k in range(KT)]
                for k in range(KT):
                    nc.sync.dma_start(r_t[k][:], R[k * P:(k + 1) * P, cs])

                hn_t = []
                for m in range(MT):
                    ps_r = pg.tile([P, BCH], F32, tag="ps_r", name="ps_r")
                    ps_z = pg.tile([P, BCH], F32, tag="ps_z", name="ps_z")
                    ps_n = pg.tile([P, BCH], F32, tag="ps_n", name="ps_n")
                    ps_i = pg.tile([P, BCH], F32, tag="ps_i", name="ps_i")
                    for k in range(KT):
                        nc.tensor.matmul(ps_r[:], wg_t[k][:, m, 0:128],
                                         r_t[k][:],
                                         start=(k == 0), stop=(k == KT - 1))
                    for k in range(KT):
                        nc.tensor.matmul(ps_z[:], wg_t[k][:, m, 128:256],
                                         r_t[k][:],
                                         start=(k == 0), stop=(k == KT - 1))
                    for k in range(2, KT):
                        nc.tensor.matmul(ps_n[:], wg_t[k][:, m, 256:384],
                                         r_t[k][:],
                                         start=(k == 2), stop=(k == KT - 1))
                    for k in range(2):
                        nc.tensor.matmul(ps_i[:], wg_t[k][:, m, 256:384],
                                         r_t[k][:],
                                         start=(k == 0), stop=(k == 1))

                    r_s = tp.tile([P, BCH], F32, tag="r_s", name="r_s")
                    nc.scalar.activation(r_s[:], ps_r[:], AFT.Sigmoid,
                                         bias=bias[:, m:m + 1])
                    z_s = tp.tile([P, BCH], F32, tag="z_s", name="z_s")
                    nc.scalar.activation(z_s[:], ps_z[:], AFT.Sigmoid,
                                         bias=bias[:, 8 + m:9 + m])
                    t_s = tp.tile([P, BCH], F32, tag="t_s", name="t_s")
                    nc.vector.scalar_tensor_tensor(t_s[:], ps_n[:],
                                                   bias[:, 24 + m:25 + m], r_s[:],
                                                   op0=ALU.add, op1=ALU.mult)
                    u_s = tp.tile([P, BCH], F32, tag="u_s", name="u_s")
                    nc.vector.tensor_tensor(u_s[:], t_s[:], ps_i[:], ALU.add)
                    n_s = tp.tile([P, BCH], F32, tag="n_s", name="n_s")
                    nc.scalar.activation(n_s[:], u_s[:], AFT.Tanh,
                                         bias=bias[:, 16 + m:17 + m])
                    v_s = tp.tile([P, BCH], F32, tag="v_s", name="v_s")
                    nc.vector.tensor_tensor(v_s[:], r_t[2 + m][:].bitcast(F32), n_s[:], ALU.subtract)
                    w_s = tp.tile([P, BCH], F32, tag="w_s", name="w_s")
                    nc.vector.tensor_tensor(w_s[:], z_s[:], v_s[:], ALU.mult)
                    hn_m = hp.tile([P, BCH], F32R, tag=f"hn{m}", name=f"hn{m}")
                    nc.vector.tensor_tensor(hn_m[:], n_s[:], w_s[:], ALU.add)
                    hn_t.append(hn_m)

                for kt in range(HS // P):
                    ps_hc = po.tile([P, BCH], F32, tag="ps_hc", name="ps_hc", bufs=3)
                    for m in range(KH):
                        nc.tensor.matmul(ps_hc[:],
                                         wh_t[m][:, kt * P:(kt + 1) * P],
                                         hn_t[m][:],
                                         start=(m == 0), stop=(m == KH - 1))
                    hc_s = op_.tile([P, BCH], F32, tag="hc_s", name="hc_s")
                    nc.scalar.activation(hc_s[:], ps_hc[:], AFT.Identity,
                                         bias=bias[:, 32 + kt:33 + kt])
                    nc.sync.dma_start(hc_out[kt * P:(kt + 1) * P, cs], hc_s[:])

                ps_p = po.tile([1, BCH], F32, tag="ps_p", name="ps_p", bufs=1)
                for m in range(KH):
                    nc.tensor.matmul(ps_p[:], wpv[:, m:m + 1],
                                     hn_t[m][:],
                                     start=(m == 0), stop=(m == KH - 1))
                p_s = op_.tile([1, BCH], F32, tag="p_s", name="p_s")
                nc.scalar.activation(p_s[:], ps_p[:], AFT.Sigmoid,
                                     bias=bias[0:1, 36:37])
                nc.sync.dma_start(probs_out[0:1, cs], p_s[:])

    nc.compile()
    return nc


def get_nc():
    if "nc" not in _CACHE:
        _CACHE["nc"] = _build()
    return _CACHE["nc"]


def make_in_maps(parent_output_label, parent_h, encoding,
                 W_ih, W_hh, b_ih, b_hh, W_p, b_p, W_h, b_h):
    x = np.asarray(parent_output_label, np.float32)
    ph = np.asarray(parent_h, np.float32)
    en = np.asarray(encoding, np.float32)
    W_ih = np.asarray(W_ih, np.float32)
    W_hh = np.asarray(W_hh, np.float32)
    b_ih = np.asarray(b_ih, np.float32)
    b_hh = np.asarray(b_hh, np.float32)
    W_p = np.asarray(W_p, np.float32)
    b_p = np.asarray(b_p, np.float32)
    W_h = np.asarray(W_h, np.float32)
    b_h = np.asarray(b_h, np.float32)

    h0 = np.concatenate([ph, en], axis=1)                      # [B, H2]
    R_full = np.ascontiguousarray(
        np.concatenate([x, h0], axis=1).T)                     # [C+H2, B]

    in_maps = []
    for d in range(D):
        # [C+H2, 3*H2] -> [C+H2, 3, 8, 128] -> [C+H2, 8, 3*128]
        WgT = np.concatenate([W_ih[d].T, W_hh[d].T], axis=0)
        Wg_d = np.ascontiguousarray(
            WgT.reshape(C + H2, 3, MT, P).transpose(0, 2, 1, 3)
               .reshape(C + H2, MT, 384))
        bv = np.zeros((P, 37), np.float32)
        br = (b_ih[d] + b_hh[d]).reshape(3, MT, P)
        bv[:, 0:8] = br[0].T           # r bias per m
        bv[:, 8:16] = br[1].T          # z bias per m
        bv[:, 16:24] = b_ih[d][2 * H2:].reshape(MT, P).T   # i_n bias
        bv[:, 24:32] = b_hh[d][2 * H2:].reshape(MT, P).T   # h_n bias
        bv[:, 32:36] = b_h[d].reshape(HS // P, P).T        # hc bias per kt
        bv[0, 36] = b_p[d]
        in_maps.append({
            "R": R_full,
            "Wg": Wg_d,
            "Wh2": np.ascontiguousarray(W_h[d].T),
            "Wp2": np.ascontiguousarray(W_p[d].reshape(KH, P).T),
            "Bv": bv,
        })
    return in_maps


def gather(results):
    hc_sum = np.zeros((HS, B), np.float32)
    probs_cols = []
    for r in results:
        hc_sum += r["hc_out"]
        probs_cols.append(r["probs_out"][0])
    h = np.tanh(hc_sum.T)
    probs = np.stack(probs_cols, axis=1)
    return np.ascontiguousarray(h, np.float32), np.ascontiguousarray(probs, np.float32)


def kernel(parent_output_label, parent_h, encoding,
           W_ih, W_hh, b_ih, b_hh, W_p, b_p, W_h, b_h):
    from concourse import bass_utils
    nc = get_nc()
    in_maps = make_in_maps(parent_output_label, parent_h, encoding,
                           W_ih, W_hh, b_ih, b_hh, W_p, b_p, W_h, b_h)
    res = bass_utils.run_bass_kernel_spmd(nc, in_maps, core_ids=list(range(D)))
    return gather(res.results)


# revision 7
# speedup vs baseline: 58.8257x; 58.8257x over previous
"""Trainium2 Bass kernel for nn_NaryDecoderCell (batched per-degree GRU decoder).

Sharding: expert-parallel over the degree dim D=8 -> one degree slot per
NeuronCore. Each core keeps its degree's weights resident in SBUF and streams
the full batch through. Host-side unshard: probs columns are gathered, and the
per-degree hc partial sums are reduced + tanh'd.

Math per degree d (reference semantics):
  h0 = concat(parent_h, encoding)                  [B, 1024]
  gi = x @ W_ih[d].T + b_ih[d]                     [B, 3072]
  gh = h0 @ W_hh[d].T + b_hh[d]                    [B, 3072]
  r = sigmoid(gi_r + gh_r); z = sigmoid(gi_z + gh_z)
  n = tanh(gi_n + r * gh_n)
  hn = (1-z)*n + z*h0
  probs_d = sigmoid(hn @ W_p[d] + b_p[d])          [B]
  hc_d = hn @ W_h[d].T + b_h[d]                    [B, 512]
Output: h = tanh(sum_d hc_d) [B, 512], probs [B, 8].

Device layout: everything is kept transposed (feature dim on partitions,
batch on the free axis) so both matmul chains contract along partitions.
The rhs stack R = concat(xT, h0T) [1280, B] gives 10 uniform K-tiles; gate
weights are host-reordered to [1280, 8(m), 384(r|z|n)] so that the r/z gates
accumulate gi+gh directly in PSUM and the per-m slices are contiguous.
Matmuls run as float32r (full PE rate at N>=256, fp32 data).
"""

import sys

sys.path.insert(0, "/opt/trn_rl_repo")

import numpy as np

B = 2048
C = 256
HS = 512
H2 = 2 * HS          # 1024
D = 8
P = 128
BCH = 256            # batch chunk (matmul moving free dim)
NCH = B // BCH       # 8
KT = (C + H2) // P   # 10 rhs K-tiles (2 from xT, 8 from h0T)
KH = H2 // P         # 8  hn K-tiles
MT = H2 // P         # 8  gate row tiles per gate

_CACHE = {}


def _build(reps=1, hw_loop=0):
    import contextlib
    import concourse.bacc as bacc
    import concourse.mybir as mybir
    from concourse.tile import TileContext

    F32 = mybir.dt.float32
    F32R = mybir.dt.float32r
    AFT = mybir.ActivationFunctionType
    ALU = mybir.AluOpType

    nc = bacc.Bacc("TRN2", target_bir_lowering=False, debug=False,
                   enable_asserts=False, num_devices=D)

    R = nc.dram_tensor("R", [C + H2, B], F32R, kind="ExternalInput").ap()
    Wg = nc.dram_tensor("Wg", [C + H2, MT, 384], F32R, kind="ExternalInput").ap()
    Wh2 = nc.dram_tensor("Wh2", [H2, HS], F32R, kind="ExternalInput").ap()
    Wp2 = nc.dram_tensor("Wp2", [P, KH], F32R, kind="ExternalInput").ap()
    Bv = nc.dram_tensor("Bv", [P, 37], F32, kind="ExternalInput").ap()
    hc_out = nc.dram_tensor("hc_out", [HS, B], F32, kind="ExternalOutput").ap()
    probs_out = nc.dram_tensor("probs_out", [1, B], F32, kind="ExternalOutput").ap()

    with TileContext(nc) as tc:
        with (
            tc.tile_pool(name="w", bufs=1) as wp,
            tc.tile_pool(name="acts", bufs=2) as ap_,
            tc.tile_pool(name="hnp", bufs=1) as hp,
            tc.tile_pool(name="tmp", bufs=2) as tp,
            tc.tile_pool(name="outs", bufs=3) as op_,
            tc.tile_pool(name="pg", bufs=1, space="PSUM") as pg,
            tc.tile_pool(name="po", bufs=1, space="PSUM") as po,
        ):
          for _rep in range(reps):
           with (tc.For_i(0, hw_loop, 1) if hw_loop else contextlib.nullcontext()):
            # ---- resident weights ----
            wg_t = [wp.tile([P, MT, 384], F32R, tag=f"wg{k}", name=f"wg{k}")
                    for k in range(KT)]
            wh_t = [wp.tile([P, HS], F32R, tag=f"wh{k}", name=f"wh{k}")
                    for k in range(KH)]
            wpv = wp.tile([P, KH], F32R, name="wpv")
            bias = wp.tile([P, 37], F32, name="bias")
            nc.sync.dma_start(bias[:], Bv[:])
            nc.sync.dma_start(wpv[:], Wp2[:])
            # m-major so the first m iteration's weights land first
            for m in range(MT):
                for k in range(KT):
                    nc.sync.dma_start(wg_t[k][:, m, :], Wg[k * P:(k + 1) * P, m, :])
            for k in range(KH):
                nc.sync.dma_start(wh_t[k][:], Wh2[k * P:(k + 1) * P, :])

            for ch in range(NCH):
                cs = slice(ch * BCH, (ch + 1) * BCH)
                r_t = [ap_.tile([P, BCH], F32R, tag=f"rhs{k}", name=f"rhs{k}")
                       for k in range(KT)]
                for k in range(KT):
                    nc.sync.dma_start(r_t[k][:], R[k * P:(k + 1) * P, cs])

                hn_t = []
                for m in range(MT):
                    ps_r = pg.tile([P, BCH], F32, tag="ps_r", name="ps_r")
                    ps_z = pg.tile([P, BCH], F32, tag="ps_z", name="ps_z")
                    ps_n = pg.tile([P, BCH], F32, tag="ps_n", name="ps_n")
                    ps_i = pg.tile([P, BCH], F32, tag="ps_i", name="ps_i")
                    for k in range(KT):
                        nc.tensor.matmul(ps_r[:], wg_t[k][:, m, 0:128],
                                         r_t[k][:],
                                         start=(k == 0), stop=(k == KT - 1))
                    for k in range(KT):
                        nc.tensor.matmul(ps_z[:], wg_t[k][:, m, 128:256],
                                         r_t[k][:],
                                         start=(k == 0), stop=(k == KT - 1))
                    for k in range(2, KT):
                        nc.tensor.matmul(ps_n[:], wg_t[k][:, m, 256:384],
                                         r_t[k][:],
                                         start=(k == 2), stop=(k == KT - 1))
                    for k in range(2):
                        nc.tensor.matmul(ps_i[:], wg_t[k][:, m, 256:384],
                                         r_t[k][:],
                                         start=(k == 0), stop=(k == 1))

                    r_s = tp.tile([P, BCH], F32, tag="r_s", name="r_s")
                    nc.scalar.activation(r_s[:], ps_r[:], AFT.Sigmoid,
                                         bias=bias[:, m:m + 1])
                    z_s = tp.tile([P, BCH], F32, tag="z_s", name="z_s")
                    nc.scalar.activation(z_s[:], ps_z[:], AFT.Sigmoid,
                                         bias=bias[:, 8 + m:9 + m])
                    t_s = tp.tile([P, BCH], F32, tag="t_s", name="t_s")
                    nc.vector.scalar_tensor_tensor(t_s[:], ps_n[:],
                                                   bias[:, 24 + m:25 + m], r_s[:],
                                                   op0=ALU.add, op1=ALU.mult)
                    u_s = tp.tile([P, BCH], F32, tag="u_s", name="u_s")
                    nc.vector.tensor_tensor(u_s[:], t_s[:], ps_i[:], ALU.add)
                    n_s = tp.tile([P, BCH], F32, tag="n_s", name="n_s")
                    nc.scalar.activation(n_s[:], u_s[:], AFT.Tanh,
                                         bias=bias[:, 16 + m:17 + m])
                    v_s = tp.tile([P, BCH], F32, tag="v_s", name="v_s")
                    nc.vector.tensor_tensor(v_s[:], r_t[2 + m][:].bitcast(F32), n_s[:], ALU.subtract)
                    w_s = tp.tile([P, BCH], F32, tag="w_s", name="w_s")
                    nc.vector.tensor_tensor(w_s[:], z_s[:], v_s[:], ALU.mult)
                    hn_m = hp.tile([P, BCH], F32R, tag=f"hn{m}", name=f"hn{m}")
                    nc.vector.tensor_tensor(hn_m[:], n_s[:], w_s[:], ALU.add)
                    hn_t.append(hn_m)

                for kt in range(HS // P):
                    ps_hc = po.tile([P, BCH], F32, tag="ps_hc", name="ps_hc", bufs=3)
                    for m in range(KH):
                        nc.tensor.matmul(ps_hc[:],
                                         wh_t[m][:, kt * P:(kt + 1) * P],
                                         hn_t[m][:],
                                         start=(m == 0), stop=(m == KH - 1))
                    hc_s = op_.tile([P, BCH], F32, tag="hc_s", name="hc_s")
                    nc.scalar.activation(hc_s[:], ps_hc[:], AFT.Identity,
                                         bias=bias[:, 32 + kt:33 + kt])
                    nc.sync.dma_start(hc_out[kt * P:(kt + 1) * P, cs], hc_s[:])

                ps_p = po.tile([1, BCH], F32, tag="ps_p", name="ps_p", bufs=1)
                for m in range(KH):
                    nc.tensor.matmul(ps_p[:], wpv[:, m:m + 1],
                                     hn_t[m][:],
                                     start=(m == 0), stop=(m == KH - 1))
                p_s = op_.tile([1, BCH], F32, tag="p_s", name="p_s")
                nc.scalar.activation(p_s[:], ps_p[:], AFT.Sigmoid,
                                     bias=bias[0:1, 36:37])
                nc.sync.dma_start(probs_out[0:1, cs], p_s[:])

    nc.compile()
    return nc


def get_nc(reps=1, hw_loop=0):
    key = ("nc", reps, hw_loop)
    if key not in _CACHE:
        _CACHE[key] = _build(reps, hw_loop)
    return _CACHE[key]


def make_in_maps(parent_output_label, parent_h, encoding,
                 W_ih, W_hh, b_ih, b_hh, W_p, b_p, W_h, b_h):
    x = np.asarray(parent_output_label, np.float32)
    ph = np.asarray(parent_h, np.float32)
    en = np.asarray(encoding, np.float32)
    W_ih = np.asarray(W_ih, np.float32)
    W_hh = np.asarray(W_hh, np.float32)
    b_ih = np.asarray(b_ih, np.float32)
    b_hh = np.asarray(b_hh, np.float32)
    W_p = np.asarray(W_p, np.float32)
    b_p = np.asarray(b_p, np.float32)
    W_h = np.asarray(W_h, np.float32)
    b_h = np.asarray(b_h, np.float32)

    h0 = np.concatenate([ph, en], axis=1)                      # [B, H2]
    R_full = np.ascontiguousarray(
        np.concatenate([x, h0], axis=1).T)                     # [C+H2, B]

    in_maps = []
    for d in range(D):
        # [C+H2, 3*H2] -> [C+H2, 3, 8, 128] -> [C+H2, 8, 3*128]
        WgT = np.concatenate([W_ih[d].T, W_hh[d].T], axis=0)
        Wg_d = np.ascontiguousarray(
            WgT.reshape(C + H2, 3, MT, P).transpose(0, 2, 1, 3)
               .reshape(C + H2, MT, 384))
        bv = np.zeros((P, 37), np.float32)
        br = (b_ih[d] + b_hh[d]).reshape(3, MT, P)
        bv[:, 0:8] = br[0].T           # r bias per m
        bv[:, 8:16] = br[1].T          # z bias per m
        bv[:, 16:24] = b_ih[d][2 * H2:].reshape(MT, P).T   # i_n bias
        bv[:, 24:32] = b_hh[d][2 * H2:].reshape(MT, P).T   # h_n bias
        bv[:, 32:36] = b_h[d].reshape(HS // P, P).T        # hc bias per kt
        bv[0, 36] = b_p[d]
        in_maps.append({
            "R": R_full,
            "Wg": Wg_d,
            "Wh2": np.ascontiguousarray(W_h[d].T),
            "Wp2": np.ascontiguousarray(W_p[d].reshape(KH, P).T),
            "Bv": bv,
        })
    return in_maps


def gather(results):
    hc_sum = np.zeros((HS, B), np.float32)
    probs_cols = []
    for r in results:
        hc_sum += r["hc_out"]
        probs_cols.append(r["probs_out"][0])
    h = np.tanh(hc_sum.T)
    probs = np.stack(probs_cols, axis=1)
    return np.ascontiguousarray(h, np.float32), np.ascontiguousarray(probs, np.float32)


def kernel(parent_output_label, parent_h, encoding,
           W_ih, W_hh, b_ih, b_hh, W_p, b_p, W_h, b_h):
    from concourse import bass_utils
    nc = get_nc()
    in_maps = make_in_maps(parent_output_label, parent_h, encoding,
                           W_ih, W_hh, b_ih, b_hh, W_p, b_p, W_h, b_h)
    res = bass_utils.run_bass_kernel_spmd(nc, in_maps, core_ids=list(range(D)))
    return gather(res.results)
